# revision 14
# baseline (speedup 1.0000x reference)
"""Trainium2 Bass kernel for GQA multi-head attention (B=2, S=2048, HID=2048,
NH=32, NKV=8, HD=64), tensor-parallel over kv heads across 8 NeuronCores.

Each core c computes q-heads [4c, 4c+4) with kv-head c against the full input,
produces a partial output y_c = O_c @ Wo_c.T; the host sums the 8 partials.

Causal path (the common case): bf16 datapath with f32 PSUM accumulation,
fused per-i-tile pipeline (project+RoPE -> attention -> output projection),
rowsum broadcast obtained for free via ones-columns appended to V, and
causal diagonal trimming of the attention matmuls.  General (arbitrary mask)
path: f32r fallback kernel.
"""

import sys

for _p in ("/opt/trn_rl_repo", "/root/.axon_site/_ro/trn_rl_repo"):
    if _p not in sys.path:
        sys.path.insert(0, _p)

import numpy as np

B, S, HID = 2, 2048, 2048
NH, NKV, HD = 32, 8, 64
SCALE = HD ** -0.5
NCORES = 8
NHC = NH // NCORES          # q heads per core (4)
BS = B * S                  # 4096
KT = HID // 128             # 16 contraction tiles for projections
IT = 512                    # attention i-tile width (q positions)
JT = 128                    # attention j-tile width (k positions)
NII = S // IT               # i tiles per batch (4)
NJ = S // JT                # j tiles per batch (16)
XCHUNK = 256                # general-path x^T chunk width
NEG = -1e9

_programs = {}


def _bf16():
    import ml_dtypes
    return ml_dtypes.bfloat16


# --------------------------------------------------------------------------
# causal path: bf16 fused kernel
# --------------------------------------------------------------------------

def _build_causal(repeat=1, timing=False):
    """bf16 causal GQA kernel, fused per-i-tile pipeline."""
    import contextlib
    import concourse.bacc as bacc
    import concourse.tile as tile
    import concourse.mybir as mybir

    f32 = mybir.dt.float32
    bf16 = mybir.dt.bfloat16
    EXP = mybir.ActivationFunctionType.Exp

    nc = bacc.Bacc("TRN2", target_bir_lowering=False, debug=False)

    xT = nc.dram_tensor("xT", [HID, BS], bf16, kind="ExternalInput").ap()
    wqT = nc.dram_tensor("wqT", [HID, NHC * HD], bf16, kind="ExternalInput").ap()
    wkvT = nc.dram_tensor("wkvT", [HID, 2 * HD], bf16, kind="ExternalInput").ap()
    woT = nc.dram_tensor("woT", [NHC * HD, HID], bf16, kind="ExternalInput").ap()
    cosQ = nc.dram_tensor("cosQ", [128, BS], bf16, kind="ExternalInput").ap()
    sinQ = nc.dram_tensor("sinQ", [128, BS], bf16, kind="ExternalInput").ap()
    ident = nc.dram_tensor("ident", [128, 64], bf16, kind="ExternalInput").ap()
    identf = nc.dram_tensor("identf", [JT, JT], bf16, kind="ExternalInput").ap()
    maskadd = nc.dram_tensor("maskadd", [JT, JT], bf16, kind="ExternalInput").ap()
    if timing:
        y = None
        ytiny = nc.dram_tensor("ytiny", [1, 8], f32, kind="ExternalOutput").ap()
    else:
        y = nc.dram_tensor("y", [BS, HID], bf16, kind="ExternalOutput").ap()

    xTr = xT.rearrange("(kt p) m -> p kt m", p=128)      # [128, KT, BS]
    wqTr = wqT.rearrange("(kt p) n -> p kt n", p=128)    # [128, KT, 256]
    wkvTr = wkvT.rearrange("(kt p) n -> p kt n", p=128)  # [128, KT, 128]
    woTr = woT.rearrange("(kt p) n -> p kt n", p=128)    # [128, 2, HID]

    with tile.TileContext(nc) as tc:
        with contextlib.ExitStack() as ctx:
            singles = ctx.enter_context(tc.tile_pool(name="singles", bufs=1))
            xpool = ctx.enter_context(tc.tile_pool(name="xpool", bufs=3))
            cospool = ctx.enter_context(tc.tile_pool(name="cospool", bufs=2))
            kvpool = ctx.enter_context(tc.tile_pool(name="kvpool", bufs=2))
            qpool = ctx.enter_context(tc.tile_pool(name="qpool", bufs=2))
            rpool = ctx.enter_context(tc.tile_pool(name="rpool", bufs=3))
            ptpool = ctx.enter_context(tc.tile_pool(name="ptpool", bufs=4))
            opool = ctx.enter_context(tc.tile_pool(name="opool", bufs=2))
            otpool = ctx.enter_context(tc.tile_pool(name="otpool", bufs=2))
            ypool = ctx.enter_context(tc.tile_pool(name="ypool", bufs=3))
            psA = ctx.enter_context(tc.tile_pool(name="psA", bufs=2, space="PSUM"))
            psS = ctx.enter_context(tc.tile_pool(name="psS", bufs=2, space="PSUM"))
            psO = ctx.enter_context(tc.tile_pool(name="psO", bufs=2, space="PSUM"))
            psY = ctx.enter_context(tc.tile_pool(name="psY", bufs=2, space="PSUM"))
            if timing:
                ydram = ctx.enter_context(
                    tc.tile_pool(name="ydram", bufs=1, space="DRAM"))
                y_scratch = ydram.tile([BS, HID], bf16)
                yt_s = None

            # ---- persistent weights / constants ----
            wq_s = singles.tile([128, KT, NHC * HD], bf16)
            nc.sync.dma_start(out=wq_s[:, 0:KT // 2], in_=wqTr[:, 0:KT // 2])
            nc.sync.dma_start(out=wq_s[:, KT // 2:KT], in_=wqTr[:, KT // 2:KT])
            wkv_s = singles.tile([128, KT, 2 * HD], bf16)
            wo_s = singles.tile([128, 2, HID], bf16)
            id_s = singles.tile([128, 64], bf16)
            idf_s = singles.tile([JT, JT], bf16)
            mask_s = singles.tile([JT, JT], bf16)

            _yt = [None]

            def emit_oproj(ot, base, blk):
                """One 128-row output-projection chunk for a finished i-tile."""
                m0 = base + blk * 128
                ys = ypool.tile([128, HID], bf16, tag="ys")
                for n4 in range(HID // IT):
                    p_y = psY.tile([128, IT], f32, tag="py")
                    for kt2 in range(2):
                        nc.tensor.matmul(
                            p_y,
                            ot[:, kt2, blk * 128:(blk + 1) * 128],
                            wo_s[:, kt2, n4 * IT:(n4 + 1) * IT],
                            start=(kt2 == 0), stop=(kt2 == 1))
                    nc.vector.tensor_copy(
                        ys[:, n4 * IT:(n4 + 1) * IT], p_y)
                ytgt = y_scratch if timing else y
                nc.sync.dma_start(out=ytgt[m0:m0 + 128, :], in_=ys)
                if timing and _yt[0] is None:
                    _yt[0] = ypool.tile([1, 8], f32, tag="yt", name="yt_s")
                    nc.vector.tensor_copy(_yt[0], ys[0:1, 0:8])
                    nc.sync.dma_start(out=ytiny, in_=_yt[0])

            pend_C = None
            for rep in range(repeat):
              for b in range(B):
                cb = b * S
                xt0 = xpool.tile([128, KT, IT], bf16, tag="xt")
                nc.sync.dma_start(
                    out=xt0[:, 0:KT // 2], in_=xTr[:, 0:KT // 2, cb:cb + IT])
                nc.sync.dma_start(
                    out=xt0[:, KT // 2:KT], in_=xTr[:, KT // 2:KT, cb:cb + IT])
                if rep == 0 and b == 0:
                    nc.sync.dma_start(out=wkv_s, in_=wkvTr)
                cq_s = cospool.tile([128, S], bf16, tag="cq")
                sq_s = cospool.tile([128, S], bf16, tag="sq")
                nc.sync.dma_start(out=cq_s, in_=cosQ[:, cb:cb + S])
                nc.sync.dma_start(out=sq_s, in_=sinQ[:, cb:cb + S])
                if rep == 0 and b == 0:
                    nc.sync.dma_start(out=id_s, in_=ident)
                    nc.sync.dma_start(out=idf_s, in_=identf)
                    nc.sync.dma_start(out=mask_s, in_=maskadd)
                    nc.sync.dma_start(out=wo_s, in_=woTr)

                # per-batch K (by i-tile) and V (seq-major with ones cols)
                kts = [kvpool.tile([64, IT], bf16, tag=f"kT{i}",
                                   name=f"kT{i}_{b}_{rep}") for i in range(NII)]
                vt = kvpool.tile([128, NJ, JT], bf16, tag="vt",
                                 name=f"vt_{b}_{rep}")
                nc.vector.memset(vt[:, :, HD:JT], 1.0)

                for ii in range(NII):
                    i0 = ii * IT
                    isl = slice(i0, i0 + IT)
                    if ii == 0:
                        xt = xt0
                    else:
                        xt = xpool.tile([128, KT, IT], bf16, tag="xt")
                        nc.sync.dma_start(
                            out=xt[:, 0:KT // 2],
                            in_=xTr[:, 0:KT // 2, cb + i0:cb + i0 + IT])
                        nc.sync.dma_start(
                            out=xt[:, KT // 2:KT],
                            in_=xTr[:, KT // 2:KT, cb + i0:cb + i0 + IT])

                    qt = qpool.tile([64, NHC, IT], bf16, tag="qt")

                    # -- Q projection + RoPE (2 groups of 2 heads) --
                    for ni in range(2):
                        p_q = psA.tile([128, IT], f32, tag="pa")
                        for kt in range(KT):
                            nc.tensor.matmul(
                                p_q, wq_s[:, kt, ni * 128:(ni + 1) * 128],
                                xt[:, kt, :],
                                start=(kt == 0), stop=(kt == KT - 1))
                        q_sb = rpool.tile([128, IT], bf16, tag="qraw")
                        nc.scalar.copy(q_sb, p_q)
                        t_c = rpool.tile([128, IT], bf16, tag="tc")
                        t_s = rpool.tile([128, IT], bf16, tag="ts")
                        nc.vector.tensor_mul(t_c, q_sb, cq_s[:, isl])
                        for r0 in (0, 64):
                            nc.vector.tensor_mul(
                                t_s[r0:r0 + 32], q_sb[r0 + 32:r0 + 64],
                                sq_s[r0 + 32:r0 + 64, isl])
                            nc.vector.tensor_mul(
                                t_s[r0 + 32:r0 + 64], q_sb[r0:r0 + 32],
                                sq_s[r0:r0 + 32, isl])
                        nc.vector.tensor_add(
                            qt[:, 2 * ni, :], t_c[0:64], t_s[0:64])
                        nc.vector.tensor_add(
                            qt[:, 2 * ni + 1, :], t_c[64:128], t_s[64:128])

                    # -- K/V projection; K RoPE; V transpose --
                    p_kv = psA.tile([128, IT], f32, tag="pa")
                    for kt in range(KT):
                        nc.tensor.matmul(
                            p_kv, wkv_s[:, kt, :], xt[:, kt, :],
                            start=(kt == 0), stop=(kt == KT - 1))
                    kv_sb = rpool.tile([128, IT], bf16, tag="qraw")
                    nc.scalar.copy(kv_sb, p_kv)
                    t_c = rpool.tile([128, IT], bf16, tag="tc")
                    t_s = rpool.tile([128, IT], bf16, tag="ts")
                    nc.vector.tensor_mul(t_c[0:64], kv_sb[0:64], cq_s[0:64, isl])
                    nc.vector.tensor_mul(
                        t_s[0:32], kv_sb[32:64], sq_s[32:64, isl])
                    nc.vector.tensor_mul(
                        t_s[32:64], kv_sb[0:32], sq_s[0:32, isl])
                    nc.vector.tensor_add(kts[ii][:, :], t_c[0:64], t_s[0:64])
                    for j2 in range(IT // JT):
                        p_v = psO.tile([128, 64], bf16, tag="po")
                        nc.tensor.transpose(
                            p_v, kv_sb[64:128, j2 * JT:(j2 + 1) * JT],
                            id_s[64:128, :])
                        nc.vector.tensor_copy(
                            vt[:, (i0 // JT) + j2, 0:HD], p_v)

                    # -- attention for this i-tile, with the previous
                    #    i-tile's output projection interleaved per head to
                    #    fill PE stalls at head transitions --
                    ot = otpool.tile([128, 2, IT], bf16, tag="ot")
                    jmax = 4 * ii + 3
                    for h in range(NHC):
                        p_o = psO.tile([128, IT], f32, tag="po")
                        for J in range(jmax + 1):
                            Jii, Jr = J // 4, J % 4
                            r = J - 4 * ii
                            c0 = r * JT if r > 0 else 0
                            p_s = psS.tile([128, IT], f32, tag="ps")
                            nc.tensor.matmul(
                                p_s[:, c0:IT],
                                kts[Jii][:, Jr * JT:(Jr + 1) * JT],
                                qt[:, h, c0:IT],
                                start=True, stop=(r < 0),
                                skip_group_check=True)
                            if r >= 0:
                                nc.tensor.matmul(
                                    p_s[:, c0:c0 + JT], idf_s, mask_s,
                                    start=False, stop=True,
                                    skip_group_check=True)
                            pt = ptpool.tile([128, IT], bf16, tag="pt")
                            nc.scalar.activation(
                                pt[:, c0:IT], p_s[:, c0:IT], EXP)
                            nc.tensor.matmul(
                                p_o[:, c0:IT], vt[:, J, :], pt[:, c0:IT],
                                start=(J == 0), stop=(J == jmax),
                                skip_group_check=True)
                        # normalize: rows 64:128 of p_o are the rowsum
                        # (broadcast via the ones columns of vt)
                        osb = opool.tile([128, IT], bf16, tag="osb")
                        nc.vector.tensor_copy(osb, p_o)
                        rcr = opool.tile([64, IT], bf16, tag="rcr")
                        with nc.allow_low_precision(reason="bf16 softmax recip"):
                            nc.vector.reciprocal(rcr, osb[64:128])
                        ntile, hr = h // 2, (h % 2) * 64
                        nc.vector.tensor_mul(
                            ot[hr:hr + 64, ntile, :], osb[0:HD], rcr)
                        if pend_C is not None:
                            emit_oproj(pend_C[0], pend_C[1], h)
                    pend_C = (ot, cb + i0)

            if pend_C is not None:
                for blk in range(IT // 128):
                    emit_oproj(pend_C[0], pend_C[1], blk)

    nc.compile()
    return nc


# --------------------------------------------------------------------------
# general (arbitrary mask) fallback: f32r kernel
# --------------------------------------------------------------------------

def _build_general(repeat=1, timing=False):
    import contextlib
    import concourse.bacc as bacc
    import concourse.tile as tile
    import concourse.mybir as mybir

    f32 = mybir.dt.float32
    f32r = mybir.dt.float32r
    EXP = mybir.ActivationFunctionType.Exp

    nc = bacc.Bacc("TRN2", target_bir_lowering=False, debug=False)

    xT = nc.dram_tensor("xT", [HID, BS], f32, kind="ExternalInput").ap()
    wqT = nc.dram_tensor("wqT", [HID, NHC * HD], f32, kind="ExternalInput").ap()
    wkvT = nc.dram_tensor("wkvT", [HID, 2 * HD], f32, kind="ExternalInput").ap()
    woT = nc.dram_tensor("woT", [NHC * HD, HID], f32, kind="ExternalInput").ap()
    cosQ = nc.dram_tensor("cosQ", [128, BS], f32, kind="ExternalInput").ap()
    sinQ = nc.dram_tensor("sinQ", [128, BS], f32, kind="ExternalInput").ap()
    identhi = nc.dram_tensor("identhi", [128, 64], f32, kind="ExternalInput").ap()
    onesd = nc.dram_tensor("onesd", [128, 64], f32, kind="ExternalInput").ap()
    maskT = nc.dram_tensor("maskT", [S, BS], f32, kind="ExternalInput").ap()
    maskTr = maskT.rearrange("(J p) i -> p J i", p=128)
    if timing:
        y = None
        ytiny = nc.dram_tensor("ytiny", [1, 8], f32, kind="ExternalOutput").ap()
    else:
        y = nc.dram_tensor("y", [BS, HID], f32, kind="ExternalOutput").ap()

    xTr = xT.rearrange("(kt p) m -> p kt m", p=128)
    wqTr = wqT.rearrange("(kt p) n -> p kt n", p=128)
    wkvTr = wkvT.rearrange("(kt p) n -> p kt n", p=128)
    woTr = woT.rearrange("(kt p) n -> p kt n", p=128)

    NMI = S // XCHUNK

    with tile.TileContext(nc) as tc:
        with contextlib.ExitStack() as ctx:
            singles = ctx.enter_context(tc.tile_pool(name="singles", bufs=1))
            xpool = ctx.enter_context(tc.tile_pool(name="xpool", bufs=3))
            cospool = ctx.enter_context(tc.tile_pool(name="cospool", bufs=1))
            qkv = ctx.enter_context(tc.tile_pool(name="qkv", bufs=1))
            ropetmp = ctx.enter_context(tc.tile_pool(name="ropetmp", bufs=2))
            ppool = ctx.enter_context(tc.tile_pool(name="ppool", bufs=3))
            nrm = ctx.enter_context(tc.tile_pool(name="nrm", bufs=2))
            ypool = ctx.enter_context(tc.tile_pool(name="ypool", bufs=2))
            mpool = ctx.enter_context(tc.tile_pool(name="mpool", bufs=2))
            pa_ps = ctx.enter_context(tc.tile_pool(name="pa_ps", bufs=2, space="PSUM"))
            mm_ps = ctx.enter_context(tc.tile_pool(name="mm_ps", bufs=2, space="PSUM"))
            o_ps = ctx.enter_context(tc.tile_pool(name="o_ps", bufs=2, space="PSUM"))
            v_ps = ctx.enter_context(tc.tile_pool(name="v_ps", bufs=1, space="PSUM"))
            b_ps = ctx.enter_context(tc.tile_pool(name="b_ps", bufs=1, space="PSUM"))
            if timing:
                ydram = ctx.enter_context(
                    tc.tile_pool(name="ydram", bufs=1, space="DRAM"))
                y_scratch = ydram.tile([BS, HID], f32)
                yt_s = None

            wq_s = singles.tile([128, KT, NHC * HD], f32r)
            nc.sync.dma_start(out=wq_s, in_=wqTr.bitcast(f32r))
            wkv_s = singles.tile([128, KT, 2 * HD], f32r)
            nc.sync.dma_start(out=wkv_s, in_=wkvTr.bitcast(f32r))
            wo_s = singles.tile([128, 2, HID], f32r)
            nc.sync.dma_start(out=wo_s, in_=woTr.bitcast(f32r))
            ident_hi = singles.tile([128, 64], f32)
            nc.sync.dma_start(out=ident_hi, in_=identhi)
            ones_t = singles.tile([128, 64], f32r)
            nc.sync.dma_start(out=ones_t, in_=onesd.bitcast(f32r))

            for rep in range(repeat):
              for b in range(B):
                cb = b * S
                cq_s = cospool.tile([128, S], f32, tag="cq")
                sq_s = cospool.tile([128, S], f32, tag="sq")
                nc.sync.dma_start(out=cq_s, in_=cosQ[:, cb:cb + S])
                nc.sync.dma_start(out=sq_s, in_=sinQ[:, cb:cb + S])

                qT4 = [qkv.tile([64, NHC, IT], f32r, tag=f"qT{i}",
                                name=f"qT{i}_{b}") for i in range(NII)]
                kT4 = [qkv.tile([64, IT], f32r, tag=f"kT{i}", bufs=2,
                                name=f"kT{i}_{b}") for i in range(NII)]
                v4 = [qkv.tile([128, IT // JT, HD + 1], f32r, tag=f"v{i}", bufs=2,
                               name=f"v{i}_{b}") for i in range(NII)]
                for i in range(NII):
                    nc.sync.dma_start(
                        out=v4[i][:, :, HD:HD + 1],
                        in_=onesd[:, 0:IT // JT]
                        .rearrange("p (a b) -> p a b", b=1).bitcast(f32r))
                oT4 = [qkv.tile([128, 2, IT], f32r, tag=f"oT{i}",
                                name=f"oT{i}_{b}") for i in range(NII)]

                for mi in range(NMI):
                    m0 = mi * XCHUNK
                    msl = slice(m0, m0 + XCHUNK)
                    mii = m0 // IT
                    l0 = m0 % IT
                    lsl = slice(l0, l0 + XCHUNK)
                    xt = xpool.tile([128, KT, XCHUNK], f32r, tag="xt")
                    nc.sync.dma_start(
                        out=xt, in_=xTr[:, :, cb + m0:cb + m0 + XCHUNK].bitcast(f32r))

                    for ni in range(2):
                        p_q = pa_ps.tile([128, IT], f32, tag="pa")
                        for kt in range(KT):
                            nc.tensor.matmul(
                                p_q[:, :XCHUNK],
                                wq_s[:, kt, ni * 128:(ni + 1) * 128],
                                xt[:, kt, :],
                                start=(kt == 0), stop=(kt == KT - 1))
                        q_raw = ropetmp.tile([128, XCHUNK], f32, tag="qraw")
                        nc.scalar.copy(q_raw, p_q[:, :XCHUNK])
                        t_c = ropetmp.tile([128, XCHUNK], f32, tag="tc")
                        t_s = ropetmp.tile([128, XCHUNK], f32, tag="ts")
                        nc.vector.tensor_mul(t_c, q_raw, cq_s[:, msl])
                        for r0 in (0, 64):
                            nc.vector.tensor_mul(
                                t_s[r0:r0 + 32], q_raw[r0 + 32:r0 + 64],
                                sq_s[r0 + 32:r0 + 64, msl])
                            nc.vector.tensor_mul(
                                t_s[r0 + 32:r0 + 64], q_raw[r0:r0 + 32],
                                sq_s[r0:r0 + 32, msl])
                        nc.vector.tensor_add(
                            qT4[mii][:, 2 * ni, lsl], t_c[0:64], t_s[0:64])
                        nc.vector.tensor_add(
                            qT4[mii][:, 2 * ni + 1, lsl], t_c[64:128], t_s[64:128])

                    p_kv = pa_ps.tile([128, IT], f32, tag="pa")
                    for kt in range(KT):
                        nc.tensor.matmul(
                            p_kv[:, :XCHUNK], wkv_s[:, kt, :], xt[:, kt, :],
                            start=(kt == 0), stop=(kt == KT - 1))
                    kv_raw = ropetmp.tile([128, XCHUNK], f32, tag="qraw")
                    nc.scalar.copy(kv_raw, p_kv[:, :XCHUNK])
                    t_c = ropetmp.tile([128, XCHUNK], f32, tag="tc")
                    t_s = ropetmp.tile([128, XCHUNK], f32, tag="ts")
                    nc.vector.tensor_mul(t_c[0:64], kv_raw[0:64], cq_s[0:64, msl])
                    nc.vector.tensor_mul(
                        t_s[0:32], kv_raw[32:64], sq_s[32:64, msl])
                    nc.vector.tensor_mul(
                        t_s[32:64], kv_raw[0:32], sq_s[0:32, msl])
                    nc.vector.tensor_add(kT4[mii][:, lsl], t_c[0:64], t_s[0:64])
                    for jj2 in range(XCHUNK // JT):
                        jt = (l0 // JT) + jj2
                        p_v = v_ps.tile([128, 64], f32, tag="vt")
                        nc.tensor.transpose(
                            p_v, kv_raw[64:128, jj2 * JT:(jj2 + 1) * JT],
                            ident_hi[64:128, :])
                        nc.vector.tensor_copy(v4[mii][:, jt, 0:HD], p_v)

                for ii in range(NII):
                    i0 = ii * IT
                    jmax = NJ - 1
                    mk_s = mpool.tile([128, NJ, IT], f32, tag="mk")
                    nc.sync.dma_start(
                        out=mk_s, in_=maskTr[:, :, cb + i0:cb + i0 + IT])
                    for h in range(NHC):
                        p_o = o_ps.tile([HD + 1, IT], f32, tag="po")
                        for J in range(jmax + 1):
                            Jii, Jr = J // (IT // JT), J % (IT // JT)
                            ksl = kT4[Jii][:, Jr * JT:(Jr + 1) * JT]
                            pt = ppool.tile([128, IT], f32r, tag="pt")
                            p_s = mm_ps.tile([128, IT], f32, tag="mm")
                            nc.tensor.matmul(
                                p_s, ksl, qT4[ii][:, h, :],
                                start=True, stop=True)
                            nc.vector.tensor_add(p_s, p_s, mk_s[:, J, :])
                            nc.scalar.activation(pt, p_s, EXP)
                            nc.tensor.matmul(
                                p_o, v4[Jii][:, Jr, :], pt,
                                start=(J == 0), stop=(J == jmax),
                                skip_group_check=True)
                        rcr_t = nrm.tile([65, IT], f32r, tag="rcr")
                        with nc.allow_low_precision(reason="f32r rowsum recip"):
                            nc.vector.reciprocal(rcr_t[64:65, :], p_o[HD:HD + 1, :])
                        p_b = b_ps.tile([64, IT], f32, tag="pb")
                        nc.tensor.matmul(
                            p_b, ones_t[64:65, :], rcr_t[64:65, :],
                            start=True, stop=True)
                        rb_s = nrm.tile([64, IT], f32, tag="rb")
                        nc.scalar.copy(rb_s, p_b)
                        ntile, hr = h // 2, (h % 2) * 64
                        if hr == 0:
                            nc.vector.tensor_mul(
                                oT4[ii][0:64, ntile, :], p_o[0:HD, :], rb_s)
                        else:
                            otmp = nrm.tile([64, IT], f32r, tag="otmp")
                            nc.vector.tensor_mul(otmp, p_o[0:HD, :], rb_s)
                            nc.vector.tensor_copy(
                                oT4[ii][64:128, ntile, :], otmp)

                for mi2 in range(S // 128):
                    m0 = mi2 * 128
                    mii2 = m0 // IT
                    lm0 = m0 % IT
                    for nh2 in range(2):
                        ys = ypool.tile([128, HID // 2], f32, tag="ys")
                        for ni2 in range(2):
                            n0 = nh2 * (HID // 2) + ni2 * IT
                            p_y = mm_ps.tile([128, IT], f32, tag="mm")
                            for kt2 in range(2):
                                nc.tensor.matmul(
                                    p_y, oT4[mii2][:, kt2, lm0:lm0 + 128],
                                    wo_s[:, kt2, n0:n0 + IT],
                                    start=(kt2 == 0), stop=(kt2 == 1))
                            if (mi2 + ni2) % 2 == 0:
                                nc.vector.tensor_copy(
                                    ys[:, ni2 * IT:(ni2 + 1) * IT], p_y)
                            else:
                                nc.scalar.copy(ys[:, ni2 * IT:(ni2 + 1) * IT], p_y)
                        ytgt = y_scratch if timing else y
                        nc.sync.dma_start(
                            out=ytgt[cb + m0:cb + m0 + 128,
                                     nh2 * (HID // 2):(nh2 + 1) * (HID // 2)],
                            in_=ys)
                        if timing and yt_s is None:
                            yt_s = ypool.tile([1, 8], f32, tag="yt")
                            nc.vector.tensor_copy(yt_s, ys[0:1, 0:8])
                            nc.sync.dma_start(out=ytiny, in_=yt_s)

    nc.compile()
    return nc


def _build(mode, repeat=1, timing=False, phases=None):
    if mode == "causal":
        return _build_causal(repeat=repeat, timing=timing)
    return _build_general(repeat=repeat, timing=timing)


def _get_program(mode):
    if mode not in _programs:
        _programs[mode] = _build(mode)
    return _programs[mode]


# --------------------------------------------------------------------------
# host-side prep
# --------------------------------------------------------------------------

def _rope_tables(cos, sin, dtype):
    cosT = np.concatenate([cos[b].T for b in range(B)], axis=1).astype(np.float32)
    sinT = np.concatenate([sin[b].T for b in range(B)], axis=1).astype(np.float32)
    sinS = np.concatenate([sinT[0:HD // 2], -sinT[0:HD // 2]], axis=0)
    cosQ = np.ascontiguousarray(np.concatenate([cosT, cosT], axis=0)).astype(dtype)
    sinQ = np.ascontiguousarray(np.concatenate([sinS, sinS], axis=0)).astype(dtype)
    return cosQ, sinQ


def _make_in_maps_causal(inputs_f32):
    hidden_states, cos, sin, attention_mask, Wq, Wk, Wv, Wo = inputs_f32
    bf16 = _bf16()
    X = np.ascontiguousarray(
        hidden_states.reshape(BS, HID).T).astype(bf16)
    cosQ, sinQ = _rope_tables(cos, sin, bf16)
    identb = np.zeros((128, 64), dtype=np.float32)
    identb[64:128, :] = np.eye(64, dtype=np.float32)
    identb = identb.astype(bf16)
    jj = np.arange(JT, dtype=np.float32)
    madd = np.where(jj[None, :] >= jj[:, None], 0.0, NEG).astype(bf16)
    identf = np.eye(JT, dtype=np.float32).astype(bf16)
    in_maps = []
    for c in range(NCORES):
        wq_c = (Wq[c * NHC * HD:(c + 1) * NHC * HD, :] * SCALE)
        wqT = np.ascontiguousarray(wq_c.T).astype(bf16)
        wk_c = Wk[c * HD:(c + 1) * HD, :]
        wv_c = Wv[c * HD:(c + 1) * HD, :]
        wkvT = np.ascontiguousarray(
            np.concatenate([wk_c, wv_c], axis=0).T).astype(bf16)
        woT = np.ascontiguousarray(
            Wo[:, c * NHC * HD:(c + 1) * NHC * HD].T).astype(bf16)
        in_maps.append({"xT": X, "wqT": wqT, "wkvT": wkvT, "woT": woT,
                        "cosQ": cosQ, "sinQ": sinQ, "ident": identb,
                        "identf": identf, "maskadd": madd})
    return in_maps


def _make_in_maps_general(inputs_f32):
    hidden_states, cos, sin, attention_mask, Wq, Wk, Wv, Wo = inputs_f32
    f32 = np.float32
    X = np.ascontiguousarray(hidden_states.reshape(BS, HID).T).astype(f32, copy=False)
    cosQ, sinQ = _rope_tables(cos, sin, f32)
    identhi = np.zeros((128, 64), dtype=f32)
    identhi[64:128, :] = np.eye(64, dtype=f32)
    onesd = np.ones((128, 64), dtype=f32)
    mT = np.concatenate([attention_mask[b, 0].T for b in range(B)], axis=1)
    mT = np.ascontiguousarray(mT).astype(f32)
    in_maps = []
    for c in range(NCORES):
        wq_c = Wq[c * NHC * HD:(c + 1) * NHC * HD, :] * SCALE
        wqT = np.ascontiguousarray(wq_c.T.astype(f32))
        wk_c = Wk[c * HD:(c + 1) * HD, :]
        wv_c = Wv[c * HD:(c + 1) * HD, :]
        wkvT = np.ascontiguousarray(np.concatenate([wk_c, wv_c], axis=0).T.astype(f32))
        woT = np.ascontiguousarray(Wo[:, c * NHC * HD:(c + 1) * NHC * HD].T.astype(f32))
        in_maps.append({"xT": X, "wqT": wqT, "wkvT": wkvT, "woT": woT,
                        "cosQ": cosQ, "sinQ": sinQ, "identhi": identhi,
                        "onesd": onesd, "maskT": mT})
    return in_maps


def _make_in_maps(inputs_f32, causal):
    if causal:
        return _make_in_maps_causal(inputs_f32)
    return _make_in_maps_general(inputs_f32)


def _is_causal(attention_mask):
    am = np.asarray(attention_mask)
    if am.shape != (B, 1, S, S):
        return False
    tri = np.where(np.tril(np.ones((S, S), dtype=bool)),
                   np.float32(0.0), np.float32(NEG))
    return bool(np.array_equal(am[0, 0], tri) and np.array_equal(am[1, 0], tri))


def kernel(hidden_states, cos, sin, attention_mask, Wq, Wk, Wv, Wo):
    from concourse.bass_utils import run_bass_kernel_spmd

    inputs_f32 = tuple(
        np.asarray(a, dtype=np.float32)
        for a in (hidden_states, cos, sin, attention_mask, Wq, Wk, Wv, Wo))

    causal = _is_causal(inputs_f32[3])
    nc = _get_program("causal" if causal else "general")
    in_maps = _make_in_maps(inputs_f32, causal)

    res = run_bass_kernel_spmd(nc, in_maps, core_ids=list(range(NCORES)))
    acc = np.zeros((BS, HID), dtype=np.float32)
    for c in range(NCORES):
        acc += res.results[c]["y"].astype(np.float32)
    return acc.reshape(B, S, HID)


# revision 17
# speedup vs baseline: 1.0386x; 1.0386x over previous
"""Trainium2 Bass kernel for GQA multi-head attention (B=2, S=2048, HID=2048,
NH=32, NKV=8, HD=64), tensor-parallel over kv heads across 8 NeuronCores.

Each core c computes q-heads [4c, 4c+4) with kv-head c against the full input,
produces a partial output y_c = O_c @ Wo_c.T; the host sums the 8 partials.

Causal path (the common case): bf16 datapath with f32 PSUM accumulation,
fused per-i-tile pipeline (project+RoPE -> attention -> output projection),
rowsum broadcast obtained for free via ones-columns appended to V, and
causal diagonal trimming of the attention matmuls.  General (arbitrary mask)
path: f32r fallback kernel.
"""

import sys

for _p in ("/opt/trn_rl_repo", "/root/.axon_site/_ro/trn_rl_repo"):
    if _p not in sys.path:
        sys.path.insert(0, _p)

import numpy as np

B, S, HID = 2, 2048, 2048
NH, NKV, HD = 32, 8, 64
SCALE = HD ** -0.5
NCORES = 8
NHC = NH // NCORES          # q heads per core (4)
BS = B * S                  # 4096
KT = HID // 128             # 16 contraction tiles for projections
IT = 512                    # attention i-tile width (q positions)
JT = 128                    # attention j-tile width (k positions)
NII = S // IT               # i tiles per batch (4)
NJ = S // JT                # j tiles per batch (16)
XCHUNK = 256                # general-path x^T chunk width
NEG = -1e9

_programs = {}


def _bf16():
    import ml_dtypes
    return ml_dtypes.bfloat16


# --------------------------------------------------------------------------
# causal path: bf16 fused kernel
# --------------------------------------------------------------------------

def _build_causal(repeat=1, timing=False):
    """bf16 causal GQA kernel, fused per-i-tile pipeline."""
    import contextlib
    import concourse.bacc as bacc
    import concourse.tile as tile
    import concourse.mybir as mybir

    f32 = mybir.dt.float32
    bf16 = mybir.dt.bfloat16
    EXP = mybir.ActivationFunctionType.Exp

    nc = bacc.Bacc("TRN2", target_bir_lowering=False, debug=False)

    xT = nc.dram_tensor("xT", [HID, BS], bf16, kind="ExternalInput").ap()
    wqT = nc.dram_tensor("wqT", [HID, NHC * HD], bf16, kind="ExternalInput").ap()
    wkvT = nc.dram_tensor("wkvT", [HID, 2 * HD], bf16, kind="ExternalInput").ap()
    woT = nc.dram_tensor("woT", [NHC * HD, HID], bf16, kind="ExternalInput").ap()
    cosQ = nc.dram_tensor("cosQ", [128, BS], bf16, kind="ExternalInput").ap()
    sinQ = nc.dram_tensor("sinQ", [128, BS], bf16, kind="ExternalInput").ap()
    ident = nc.dram_tensor("ident", [128, 64], bf16, kind="ExternalInput").ap()
    identf = nc.dram_tensor("identf", [JT, JT], bf16, kind="ExternalInput").ap()
    maskadd = nc.dram_tensor("maskadd", [JT, JT], bf16, kind="ExternalInput").ap()
    if timing:
        y = None
        ytiny = nc.dram_tensor("ytiny", [1, 8], f32, kind="ExternalOutput").ap()
    else:
        y = nc.dram_tensor("y", [BS, HID], bf16, kind="ExternalOutput").ap()

    xTr = xT.rearrange("(kt p) m -> p kt m", p=128)      # [128, KT, BS]
    wqTr = wqT.rearrange("(kt p) n -> p kt n", p=128)    # [128, KT, 256]
    wkvTr = wkvT.rearrange("(kt p) n -> p kt n", p=128)  # [128, KT, 128]
    woTr = woT.rearrange("(kt p) n -> p kt n", p=128)    # [128, 2, HID]

    with tile.TileContext(nc) as tc:
        with contextlib.ExitStack() as ctx:
            singles = ctx.enter_context(tc.tile_pool(name="singles", bufs=1))
            xpool = ctx.enter_context(tc.tile_pool(name="xpool", bufs=3))
            cospool = ctx.enter_context(tc.tile_pool(name="cospool", bufs=2))
            kvpool = ctx.enter_context(tc.tile_pool(name="kvpool", bufs=2))
            qpool = ctx.enter_context(tc.tile_pool(name="qpool", bufs=2))
            rpool = ctx.enter_context(tc.tile_pool(name="rpool", bufs=3))
            ptpool = ctx.enter_context(tc.tile_pool(name="ptpool", bufs=4))
            opool = ctx.enter_context(tc.tile_pool(name="opool", bufs=2))
            otpool = ctx.enter_context(tc.tile_pool(name="otpool", bufs=2))
            ypool = ctx.enter_context(tc.tile_pool(name="ypool", bufs=3))
            psA = ctx.enter_context(tc.tile_pool(name="psA", bufs=2, space="PSUM"))
            psS = ctx.enter_context(tc.tile_pool(name="psS", bufs=2, space="PSUM"))
            psO = ctx.enter_context(tc.tile_pool(name="psO", bufs=2, space="PSUM"))
            psY = ctx.enter_context(tc.tile_pool(name="psY", bufs=2, space="PSUM"))
            if timing:
                ydram = ctx.enter_context(
                    tc.tile_pool(name="ydram", bufs=1, space="DRAM"))
                y_scratch = ydram.tile([BS, HID], bf16)
                yt_s = None

            # ---- persistent weights / constants ----
            wq_s = singles.tile([128, KT, NHC * HD], bf16)
            nc.sync.dma_start(out=wq_s[:, 0:KT // 2], in_=wqTr[:, 0:KT // 2])
            nc.sync.dma_start(out=wq_s[:, KT // 2:KT], in_=wqTr[:, KT // 2:KT])
            wkv_s = singles.tile([128, KT, 2 * HD], bf16)
            wo_s = singles.tile([128, 2, HID], bf16)
            id_s = singles.tile([128, 64], bf16)
            idf_s = singles.tile([JT, JT], bf16)
            mask_s = singles.tile([JT, JT], bf16)

            _yt = [None]

            def emit_oproj(ot, base, blk):
                """One 128-row output-projection chunk for a finished i-tile."""
                m0 = base + blk * 128
                ys = ypool.tile([128, HID], bf16, tag="ys")
                for n4 in range(HID // IT):
                    p_y = psY.tile([128, IT], f32, tag="py")
                    for kt2 in range(2):
                        nc.tensor.matmul(
                            p_y,
                            ot[:, kt2, blk * 128:(blk + 1) * 128],
                            wo_s[:, kt2, n4 * IT:(n4 + 1) * IT],
                            start=(kt2 == 0), stop=(kt2 == 1))
                    nc.vector.tensor_copy(
                        ys[:, n4 * IT:(n4 + 1) * IT], p_y)
                ytgt = y_scratch if timing else y
                nc.sync.dma_start(out=ytgt[m0:m0 + 128, :], in_=ys)
                if timing and _yt[0] is None:
                    _yt[0] = ypool.tile([1, 8], f32, tag="yt", name="yt_s")
                    nc.vector.tensor_copy(_yt[0], ys[0:1, 0:8])
                    nc.sync.dma_start(out=ytiny, in_=_yt[0])

            pend_C = None
            for rep in range(repeat):
              for b in range(B):
                cb = b * S
                xt0 = xpool.tile([128, KT, IT], bf16, tag="xt")
                nc.sync.dma_start(
                    out=xt0[:, 0:KT // 2], in_=xTr[:, 0:KT // 2, cb:cb + IT])
                nc.sync.dma_start(
                    out=xt0[:, KT // 2:KT], in_=xTr[:, KT // 2:KT, cb:cb + IT])
                if rep == 0 and b == 0:
                    nc.sync.dma_start(out=wkv_s, in_=wkvTr)
                cq_s = cospool.tile([128, S], bf16, tag="cq")
                sq_s = cospool.tile([128, S], bf16, tag="sq")
                nc.sync.dma_start(out=cq_s, in_=cosQ[:, cb:cb + S])
                nc.sync.dma_start(out=sq_s, in_=sinQ[:, cb:cb + S])
                if rep == 0 and b == 0:
                    nc.sync.dma_start(out=id_s, in_=ident)
                    nc.sync.dma_start(out=idf_s, in_=identf)
                    nc.sync.dma_start(out=mask_s, in_=maskadd)
                    nc.sync.dma_start(out=wo_s, in_=woTr)

                # per-batch K (by i-tile) and V (seq-major with ones cols)
                kts = [kvpool.tile([64, IT], bf16, tag=f"kT{i}",
                                   name=f"kT{i}_{b}_{rep}") for i in range(NII)]
                vt = kvpool.tile([128, NJ, JT], bf16, tag="vt",
                                 name=f"vt_{b}_{rep}")
                nc.vector.memset(vt[:, :, HD:JT], 1.0)

                for ii in range(NII):
                    i0 = ii * IT
                    isl = slice(i0, i0 + IT)
                    if ii == 0:
                        xt = xt0
                    else:
                        xt = xpool.tile([128, KT, IT], bf16, tag="xt")
                        nc.sync.dma_start(
                            out=xt[:, 0:KT // 2],
                            in_=xTr[:, 0:KT // 2, cb + i0:cb + i0 + IT])
                        nc.sync.dma_start(
                            out=xt[:, KT // 2:KT],
                            in_=xTr[:, KT // 2:KT, cb + i0:cb + i0 + IT])

                    qt = qpool.tile([64, NHC, IT], bf16, tag="qt")

                    # -- Q projection + RoPE (2 groups of 2 heads) --
                    for ni in range(2):
                        p_q = psA.tile([128, IT], f32, tag="pa")
                        for kt in range(KT):
                            nc.tensor.matmul(
                                p_q, wq_s[:, kt, ni * 128:(ni + 1) * 128],
                                xt[:, kt, :],
                                start=(kt == 0), stop=(kt == KT - 1))
                        q_sb = rpool.tile([128, IT], bf16, tag="qraw")
                        nc.scalar.copy(q_sb, p_q)
                        t_c = rpool.tile([128, IT], bf16, tag="tc")
                        t_s = rpool.tile([128, IT], bf16, tag="ts")
                        nc.vector.tensor_mul(t_c, q_sb, cq_s[:, isl])
                        for r0 in (0, 64):
                            nc.vector.tensor_mul(
                                t_s[r0:r0 + 32], q_sb[r0 + 32:r0 + 64],
                                sq_s[r0 + 32:r0 + 64, isl])
                            nc.vector.tensor_mul(
                                t_s[r0 + 32:r0 + 64], q_sb[r0:r0 + 32],
                                sq_s[r0:r0 + 32, isl])
                        nc.vector.tensor_add(
                            qt[:, 2 * ni, :], t_c[0:64], t_s[0:64])
                        nc.vector.tensor_add(
                            qt[:, 2 * ni + 1, :], t_c[64:128], t_s[64:128])

                    # -- K/V projection; K RoPE; V transpose --
                    p_kv = psA.tile([128, IT], f32, tag="pa")
                    for kt in range(KT):
                        nc.tensor.matmul(
                            p_kv, wkv_s[:, kt, :], xt[:, kt, :],
                            start=(kt == 0), stop=(kt == KT - 1))
                    kv_sb = rpool.tile([128, IT], bf16, tag="qraw")
                    nc.scalar.copy(kv_sb, p_kv)
                    t_c = rpool.tile([128, IT], bf16, tag="tc")
                    t_s = rpool.tile([128, IT], bf16, tag="ts")
                    nc.vector.tensor_mul(t_c[0:64], kv_sb[0:64], cq_s[0:64, isl])
                    nc.vector.tensor_mul(
                        t_s[0:32], kv_sb[32:64], sq_s[32:64, isl])
                    nc.vector.tensor_mul(
                        t_s[32:64], kv_sb[0:32], sq_s[0:32, isl])
                    nc.vector.tensor_add(kts[ii][:, :], t_c[0:64], t_s[0:64])
                    for j2 in range(IT // JT):
                        p_v = psO.tile([128, 64], bf16, tag="po")
                        nc.tensor.transpose(
                            p_v, kv_sb[64:128, j2 * JT:(j2 + 1) * JT],
                            id_s[64:128, :])
                        nc.vector.tensor_copy(
                            vt[:, (i0 // JT) + j2, 0:HD], p_v)

                    # -- attention for this i-tile, with the previous
                    #    i-tile's output projection interleaved per head to
                    #    fill PE stalls at head transitions --
                    ot = otpool.tile([128, 2, IT], bf16, tag="ot")
                    jmax = 4 * ii + 3
                    for h in range(NHC):
                        p_o = psO.tile([128, IT], f32, tag="po")
                        for J in range(jmax + 1):
                            Jii, Jr = J // 4, J % 4
                            r = J - 4 * ii
                            c0 = r * JT if r > 0 else 0
                            p_s = psS.tile([128, IT], f32, tag="ps")
                            nc.tensor.matmul(
                                p_s[:, c0:IT],
                                kts[Jii][:, Jr * JT:(Jr + 1) * JT],
                                qt[:, h, c0:IT],
                                start=True, stop=(r < 0),
                                skip_group_check=True)
                            if r >= 0:
                                nc.tensor.matmul(
                                    p_s[:, c0:c0 + JT], idf_s, mask_s,
                                    start=False, stop=True,
                                    skip_group_check=True)
                            pt = ptpool.tile([128, IT], bf16, tag="pt")
                            nc.scalar.activation(
                                pt[:, c0:IT], p_s[:, c0:IT], EXP)
                            nc.tensor.matmul(
                                p_o[:, c0:IT], vt[:, J, :], pt[:, c0:IT],
                                start=(J == 0), stop=(J == jmax),
                                skip_group_check=True)
                        # normalize: rows 64:128 of p_o are the rowsum
                        # (broadcast via the ones columns of vt)
                        osb = opool.tile([128, IT], bf16, tag="osb")
                        nc.vector.tensor_copy(osb, p_o)
                        rcr = opool.tile([64, IT], bf16, tag="rcr")
                        with nc.allow_low_precision(reason="bf16 softmax recip"):
                            nc.vector.reciprocal(rcr, osb[64:128])
                        ntile, hr = h // 2, (h % 2) * 64
                        nc.vector.tensor_mul(
                            ot[hr:hr + 64, ntile, :], osb[0:HD], rcr)
                        if pend_C is not None:
                            emit_oproj(pend_C[0], pend_C[1], h)
                    pend_C = (ot, cb + i0)

            if pend_C is not None:
                for blk in range(IT // 128):
                    emit_oproj(pend_C[0], pend_C[1], blk)

    nc.compile()
    return nc


# --------------------------------------------------------------------------
# general (arbitrary mask) fallback: f32r kernel
# --------------------------------------------------------------------------

def _build_general(repeat=1, timing=False):
    import contextlib
    import concourse.bacc as bacc
    import concourse.tile as tile
    import concourse.mybir as mybir

    f32 = mybir.dt.float32
    f32r = mybir.dt.float32r
    bf16 = mybir.dt.bfloat16
    EXP = mybir.ActivationFunctionType.Exp

    nc = bacc.Bacc("TRN2", target_bir_lowering=False, debug=False)

    xT = nc.dram_tensor("xT", [HID, BS], f32, kind="ExternalInput").ap()
    wqT = nc.dram_tensor("wqT", [HID, NHC * HD], f32, kind="ExternalInput").ap()
    wkvT = nc.dram_tensor("wkvT", [HID, 2 * HD], f32, kind="ExternalInput").ap()
    woT = nc.dram_tensor("woT", [NHC * HD, HID], f32, kind="ExternalInput").ap()
    cosQ = nc.dram_tensor("cosQ", [128, BS], f32, kind="ExternalInput").ap()
    sinQ = nc.dram_tensor("sinQ", [128, BS], f32, kind="ExternalInput").ap()
    identhi = nc.dram_tensor("identhi", [128, 64], f32, kind="ExternalInput").ap()
    onesd = nc.dram_tensor("onesd", [128, 64], f32, kind="ExternalInput").ap()
    maskT = nc.dram_tensor("maskT", [S, BS], bf16, kind="ExternalInput").ap()
    maskTr = maskT.rearrange("(J p) i -> p J i", p=128)
    if timing:
        y = None
        ytiny = nc.dram_tensor("ytiny", [1, 8], f32, kind="ExternalOutput").ap()
    else:
        y = nc.dram_tensor("y", [BS, HID], f32, kind="ExternalOutput").ap()

    xTr = xT.rearrange("(kt p) m -> p kt m", p=128)
    wqTr = wqT.rearrange("(kt p) n -> p kt n", p=128)
    wkvTr = wkvT.rearrange("(kt p) n -> p kt n", p=128)
    woTr = woT.rearrange("(kt p) n -> p kt n", p=128)

    NMI = S // XCHUNK

    with tile.TileContext(nc) as tc:
        with contextlib.ExitStack() as ctx:
            singles = ctx.enter_context(tc.tile_pool(name="singles", bufs=1))
            xpool = ctx.enter_context(tc.tile_pool(name="xpool", bufs=3))
            cospool = ctx.enter_context(tc.tile_pool(name="cospool", bufs=1))
            qkv = ctx.enter_context(tc.tile_pool(name="qkv", bufs=1))
            ropetmp = ctx.enter_context(tc.tile_pool(name="ropetmp", bufs=1))
            ppool = ctx.enter_context(tc.tile_pool(name="ppool", bufs=3))
            nrm = ctx.enter_context(tc.tile_pool(name="nrm", bufs=1))
            ypool = ctx.enter_context(tc.tile_pool(name="ypool", bufs=2))
            mpool = ctx.enter_context(tc.tile_pool(name="mpool", bufs=1))
            pa_ps = ctx.enter_context(tc.tile_pool(name="pa_ps", bufs=2, space="PSUM"))
            mm_ps = ctx.enter_context(tc.tile_pool(name="mm_ps", bufs=2, space="PSUM"))
            o_ps = ctx.enter_context(tc.tile_pool(name="o_ps", bufs=2, space="PSUM"))
            v_ps = ctx.enter_context(tc.tile_pool(name="v_ps", bufs=1, space="PSUM"))
            b_ps = ctx.enter_context(tc.tile_pool(name="b_ps", bufs=1, space="PSUM"))
            if timing:
                ydram = ctx.enter_context(
                    tc.tile_pool(name="ydram", bufs=1, space="DRAM"))
                y_scratch = ydram.tile([BS, HID], f32)
                yt_s = None

            wq_s = singles.tile([128, KT, NHC * HD], f32r)
            nc.sync.dma_start(out=wq_s, in_=wqTr.bitcast(f32r))
            wkv_s = singles.tile([128, KT, 2 * HD], f32r)
            nc.sync.dma_start(out=wkv_s, in_=wkvTr.bitcast(f32r))
            wo_s = singles.tile([128, 2, HID], f32r)
            nc.sync.dma_start(out=wo_s, in_=woTr.bitcast(f32r))
            ident_hi = singles.tile([128, 64], f32)
            nc.sync.dma_start(out=ident_hi, in_=identhi)
            ones_t = singles.tile([128, 64], f32r)
            nc.sync.dma_start(out=ones_t, in_=onesd.bitcast(f32r))

            for rep in range(repeat):
              for b in range(B):
                cb = b * S
                cq_s = cospool.tile([128, S], f32, tag="cq")
                sq_s = cospool.tile([128, S], f32, tag="sq")
                nc.sync.dma_start(out=cq_s, in_=cosQ[:, cb:cb + S])
                nc.sync.dma_start(out=sq_s, in_=sinQ[:, cb:cb + S])

                qT4 = [qkv.tile([64, NHC, IT], f32r, tag=f"qT{i}",
                                name=f"qT{i}_{b}") for i in range(NII)]
                kT4 = [qkv.tile([64, IT], f32r, tag=f"kT{i}", bufs=1,
                                name=f"kT{i}_{b}") for i in range(NII)]
                v4 = [qkv.tile([128, IT // JT, HD + 1], f32r, tag=f"v{i}", bufs=1,
                               name=f"v{i}_{b}") for i in range(NII)]
                for i in range(NII):
                    nc.sync.dma_start(
                        out=v4[i][:, :, HD:HD + 1],
                        in_=onesd[:, 0:IT // JT]
                        .rearrange("p (a b) -> p a b", b=1).bitcast(f32r))
                oT4 = [qkv.tile([128, 2, IT], f32r, tag=f"oT{i}",
                                name=f"oT{i}_{b}") for i in range(NII)]

                for mi in range(NMI):
                    m0 = mi * XCHUNK
                    msl = slice(m0, m0 + XCHUNK)
                    mii = m0 // IT
                    l0 = m0 % IT
                    lsl = slice(l0, l0 + XCHUNK)
                    xt = xpool.tile([128, KT, XCHUNK], f32r, tag="xt")
                    nc.sync.dma_start(
                        out=xt, in_=xTr[:, :, cb + m0:cb + m0 + XCHUNK].bitcast(f32r))

                    for ni in range(2):
                        p_q = pa_ps.tile([128, IT], f32, tag="pa")
                        for kt in range(KT):
                            nc.tensor.matmul(
                                p_q[:, :XCHUNK],
                                wq_s[:, kt, ni * 128:(ni + 1) * 128],
                                xt[:, kt, :],
                                start=(kt == 0), stop=(kt == KT - 1))
                        q_raw = ropetmp.tile([128, XCHUNK], f32, tag="qraw")
                        nc.scalar.copy(q_raw, p_q[:, :XCHUNK])
                        t_c = ropetmp.tile([128, XCHUNK], f32, tag="tc")
                        t_s = ropetmp.tile([128, XCHUNK], f32, tag="ts")
                        nc.vector.tensor_mul(t_c, q_raw, cq_s[:, msl])
                        for r0 in (0, 64):
                            nc.vector.tensor_mul(
                                t_s[r0:r0 + 32], q_raw[r0 + 32:r0 + 64],
                                sq_s[r0 + 32:r0 + 64, msl])
                            nc.vector.tensor_mul(
                                t_s[r0 + 32:r0 + 64], q_raw[r0:r0 + 32],
                                sq_s[r0:r0 + 32, msl])
                        nc.vector.tensor_add(
                            qT4[mii][:, 2 * ni, lsl], t_c[0:64], t_s[0:64])
                        nc.vector.tensor_add(
                            qT4[mii][:, 2 * ni + 1, lsl], t_c[64:128], t_s[64:128])

                    p_kv = pa_ps.tile([128, IT], f32, tag="pa")
                    for kt in range(KT):
                        nc.tensor.matmul(
                            p_kv[:, :XCHUNK], wkv_s[:, kt, :], xt[:, kt, :],
                            start=(kt == 0), stop=(kt == KT - 1))
                    kv_raw = ropetmp.tile([128, XCHUNK], f32, tag="qraw")
                    nc.scalar.copy(kv_raw, p_kv[:, :XCHUNK])
                    t_c = ropetmp.tile([128, XCHUNK], f32, tag="tc")
                    t_s = ropetmp.tile([128, XCHUNK], f32, tag="ts")
                    nc.vector.tensor_mul(t_c[0:64], kv_raw[0:64], cq_s[0:64, msl])
                    nc.vector.tensor_mul(
                        t_s[0:32], kv_raw[32:64], sq_s[32:64, msl])
                    nc.vector.tensor_mul(
                        t_s[32:64], kv_raw[0:32], sq_s[0:32, msl])
                    nc.vector.tensor_add(kT4[mii][:, lsl], t_c[0:64], t_s[0:64])
                    for jj2 in range(XCHUNK // JT):
                        jt = (l0 // JT) + jj2
                        p_v = v_ps.tile([128, 64], f32, tag="vt")
                        nc.tensor.transpose(
                            p_v, kv_raw[64:128, jj2 * JT:(jj2 + 1) * JT],
                            ident_hi[64:128, :])
                        nc.vector.tensor_copy(v4[mii][:, jt, 0:HD], p_v)

                for ii in range(NII):
                    i0 = ii * IT
                    jmax = NJ - 1
                    mk_s = mpool.tile([128, NJ, IT], bf16, tag="mk")
                    nc.sync.dma_start(
                        out=mk_s, in_=maskTr[:, :, cb + i0:cb + i0 + IT])
                    for h in range(NHC):
                        p_o = o_ps.tile([HD + 1, IT], f32, tag="po")
                        for J in range(jmax + 1):
                            Jii, Jr = J // (IT // JT), J % (IT // JT)
                            ksl = kT4[Jii][:, Jr * JT:(Jr + 1) * JT]
                            pt = ppool.tile([128, IT], f32r, tag="pt")
                            p_s = mm_ps.tile([128, IT], f32, tag="mm")
                            nc.tensor.matmul(
                                p_s, ksl, qT4[ii][:, h, :],
                                start=True, stop=True)
                            nc.vector.tensor_add(p_s, p_s, mk_s[:, J, :])
                            nc.scalar.activation(pt, p_s, EXP)
                            nc.tensor.matmul(
                                p_o, v4[Jii][:, Jr, :], pt,
                                start=(J == 0), stop=(J == jmax),
                                skip_group_check=True)
                        rcr_t = nrm.tile([65, IT], f32r, tag="rcr")
                        with nc.allow_low_precision(reason="f32r rowsum recip"):
                            nc.vector.reciprocal(rcr_t[64:65, :], p_o[HD:HD + 1, :])
                        p_b = b_ps.tile([64, IT], f32, tag="pb")
                        nc.tensor.matmul(
                            p_b, ones_t[64:65, :], rcr_t[64:65, :],
                            start=True, stop=True)
                        rb_s = nrm.tile([64, IT], f32, tag="rb")
                        nc.scalar.copy(rb_s, p_b)
                        ntile, hr = h // 2, (h % 2) * 64
                        if hr == 0:
                            nc.vector.tensor_mul(
                                oT4[ii][0:64, ntile, :], p_o[0:HD, :], rb_s)
                        else:
                            otmp = nrm.tile([64, IT], f32r, tag="otmp")
                            nc.vector.tensor_mul(otmp, p_o[0:HD, :], rb_s)
                            nc.vector.tensor_copy(
                                oT4[ii][64:128, ntile, :], otmp)

                for mi2 in range(S // 128):
                    m0 = mi2 * 128
                    mii2 = m0 // IT
                    lm0 = m0 % IT
                    for nh2 in range(2):
                        ys = ypool.tile([128, HID // 2], f32, tag="ys")
                        for ni2 in range(2):
                            n0 = nh2 * (HID // 2) + ni2 * IT
                            p_y = mm_ps.tile([128, IT], f32, tag="mm")
                            for kt2 in range(2):
                                nc.tensor.matmul(
                                    p_y, oT4[mii2][:, kt2, lm0:lm0 + 128],
                                    wo_s[:, kt2, n0:n0 + IT],
                                    start=(kt2 == 0), stop=(kt2 == 1))
                            if (mi2 + ni2) % 2 == 0:
                                nc.vector.tensor_copy(
                                    ys[:, ni2 * IT:(ni2 + 1) * IT], p_y)
                            else:
                                nc.scalar.copy(ys[:, ni2 * IT:(ni2 + 1) * IT], p_y)
                        ytgt = y_scratch if timing else y
                        nc.sync.dma_start(
                            out=ytgt[cb + m0:cb + m0 + 128,
                                     nh2 * (HID // 2):(nh2 + 1) * (HID // 2)],
                            in_=ys)
                        if timing and yt_s is None:
                            yt_s = ypool.tile([1, 8], f32, tag="yt")
                            nc.vector.tensor_copy(yt_s, ys[0:1, 0:8])
                            nc.sync.dma_start(out=ytiny, in_=yt_s)

    nc.compile()
    return nc


def _build(mode, repeat=1, timing=False, phases=None):
    if mode == "causal":
        return _build_causal(repeat=repeat, timing=timing)
    return _build_general(repeat=repeat, timing=timing)


def _get_program(mode):
    if mode not in _programs:
        _programs[mode] = _build(mode)
    return _programs[mode]


# --------------------------------------------------------------------------
# host-side prep
# --------------------------------------------------------------------------

def _rope_tables(cos, sin, dtype):
    cosT = np.concatenate([cos[b].T for b in range(B)], axis=1).astype(np.float32)
    sinT = np.concatenate([sin[b].T for b in range(B)], axis=1).astype(np.float32)
    sinS = np.concatenate([sinT[0:HD // 2], -sinT[0:HD // 2]], axis=0)
    cosQ = np.ascontiguousarray(np.concatenate([cosT, cosT], axis=0)).astype(dtype)
    sinQ = np.ascontiguousarray(np.concatenate([sinS, sinS], axis=0)).astype(dtype)
    return cosQ, sinQ


def _make_in_maps_causal(inputs_f32):
    hidden_states, cos, sin, attention_mask, Wq, Wk, Wv, Wo = inputs_f32
    bf16 = _bf16()
    X = np.ascontiguousarray(
        hidden_states.reshape(BS, HID).T).astype(bf16)
    cosQ, sinQ = _rope_tables(cos, sin, bf16)
    identb = np.zeros((128, 64), dtype=np.float32)
    identb[64:128, :] = np.eye(64, dtype=np.float32)
    identb = identb.astype(bf16)
    jj = np.arange(JT, dtype=np.float32)
    madd = np.where(jj[None, :] >= jj[:, None], 0.0, NEG).astype(bf16)
    identf = np.eye(JT, dtype=np.float32).astype(bf16)
    in_maps = []
    for c in range(NCORES):
        wq_c = (Wq[c * NHC * HD:(c + 1) * NHC * HD, :] * SCALE)
        wqT = np.ascontiguousarray(wq_c.T).astype(bf16)
        wk_c = Wk[c * HD:(c + 1) * HD, :]
        wv_c = Wv[c * HD:(c + 1) * HD, :]
        wkvT = np.ascontiguousarray(
            np.concatenate([wk_c, wv_c], axis=0).T).astype(bf16)
        woT = np.ascontiguousarray(
            Wo[:, c * NHC * HD:(c + 1) * NHC * HD].T).astype(bf16)
        in_maps.append({"xT": X, "wqT": wqT, "wkvT": wkvT, "woT": woT,
                        "cosQ": cosQ, "sinQ": sinQ, "ident": identb,
                        "identf": identf, "maskadd": madd})
    return in_maps


def _make_in_maps_general(inputs_f32):
    hidden_states, cos, sin, attention_mask, Wq, Wk, Wv, Wo = inputs_f32
    f32 = np.float32
    X = np.ascontiguousarray(hidden_states.reshape(BS, HID).T).astype(f32, copy=False)
    cosQ, sinQ = _rope_tables(cos, sin, f32)
    identhi = np.zeros((128, 64), dtype=f32)
    identhi[64:128, :] = np.eye(64, dtype=f32)
    onesd = np.ones((128, 64), dtype=f32)
    mT = np.concatenate([attention_mask[b, 0].T for b in range(B)], axis=1)
    mT = np.ascontiguousarray(mT).astype(_bf16())
    in_maps = []
    for c in range(NCORES):
        wq_c = Wq[c * NHC * HD:(c + 1) * NHC * HD, :] * SCALE
        wqT = np.ascontiguousarray(wq_c.T.astype(f32))
        wk_c = Wk[c * HD:(c + 1) * HD, :]
        wv_c = Wv[c * HD:(c + 1) * HD, :]
        wkvT = np.ascontiguousarray(np.concatenate([wk_c, wv_c], axis=0).T.astype(f32))
        woT = np.ascontiguousarray(Wo[:, c * NHC * HD:(c + 1) * NHC * HD].T.astype(f32))
        in_maps.append({"xT": X, "wqT": wqT, "wkvT": wkvT, "woT": woT,
                        "cosQ": cosQ, "sinQ": sinQ, "identhi": identhi,
                        "onesd": onesd, "maskT": mT})
    return in_maps


def _make_in_maps(inputs_f32, causal):
    if causal:
        return _make_in_maps_causal(inputs_f32)
    return _make_in_maps_general(inputs_f32)


def _is_causal(attention_mask):
    am = np.asarray(attention_mask)
    if am.shape != (B, 1, S, S):
        return False
    tri = np.where(np.tril(np.ones((S, S), dtype=bool)),
                   np.float32(0.0), np.float32(NEG))
    return bool(np.array_equal(am[0, 0], tri) and np.array_equal(am[1, 0], tri))


def kernel(hidden_states, cos, sin, attention_mask, Wq, Wk, Wv, Wo):
    from concourse.bass_utils import run_bass_kernel_spmd

    inputs_f32 = tuple(
        np.asarray(a, dtype=np.float32)
        for a in (hidden_states, cos, sin, attention_mask, Wq, Wk, Wv, Wo))

    causal = _is_causal(inputs_f32[3])
    nc = _get_program("causal" if causal else "general")
    in_maps = _make_in_maps(inputs_f32, causal)

    res = run_bass_kernel_spmd(nc, in_maps, core_ids=list(range(NCORES)))
    acc = np.zeros((BS, HID), dtype=np.float32)
    for c in range(NCORES):
        acc += res.results[c]["y"].astype(np.float32)
    return acc.reshape(B, S, HID)


# revision 19
# speedup vs baseline: 1.0487x; 1.0098x over previous
"""Trainium2 Bass kernel for GQA multi-head attention (B=2, S=2048, HID=2048,
NH=32, NKV=8, HD=64), tensor-parallel over kv heads across 8 NeuronCores.

Each core c computes q-heads [4c, 4c+4) with kv-head c against the full input,
produces a partial output y_c = O_c @ Wo_c.T; the host sums the 8 partials.

Causal path (the common case): bf16 datapath with f32 PSUM accumulation,
fused per-i-tile pipeline (project+RoPE -> attention -> output projection),
rowsum broadcast obtained for free via ones-columns appended to V, and
causal diagonal trimming of the attention matmuls.  General (arbitrary mask)
path: f32r fallback kernel.
"""

import sys

for _p in ("/opt/trn_rl_repo", "/root/.axon_site/_ro/trn_rl_repo"):
    if _p not in sys.path:
        sys.path.insert(0, _p)

import numpy as np

B, S, HID = 2, 2048, 2048
NH, NKV, HD = 32, 8, 64
SCALE = HD ** -0.5
NCORES = 8
NHC = NH // NCORES          # q heads per core (4)
BS = B * S                  # 4096
KT = HID // 128             # 16 contraction tiles for projections
IT = 512                    # attention i-tile width (q positions)
JT = 128                    # attention j-tile width (k positions)
NII = S // IT               # i tiles per batch (4)
NJ = S // JT                # j tiles per batch (16)
XCHUNK = 256                # general-path x^T chunk width
NEG = -1e9

_programs = {}


def _bf16():
    import ml_dtypes
    return ml_dtypes.bfloat16


# --------------------------------------------------------------------------
# causal path: bf16 fused kernel
# --------------------------------------------------------------------------

def _build_causal(repeat=1, timing=False):
    """bf16 causal GQA kernel, fused per-i-tile pipeline."""
    import contextlib
    import concourse.bacc as bacc
    import concourse.tile as tile
    import concourse.mybir as mybir

    f32 = mybir.dt.float32
    bf16 = mybir.dt.bfloat16
    EXP = mybir.ActivationFunctionType.Exp

    nc = bacc.Bacc("TRN2", target_bir_lowering=False, debug=False)

    xT = nc.dram_tensor("xT", [HID, BS], bf16, kind="ExternalInput").ap()
    wqT = nc.dram_tensor("wqT", [HID, NHC * HD], bf16, kind="ExternalInput").ap()
    wkvT = nc.dram_tensor("wkvT", [HID, 2 * HD], bf16, kind="ExternalInput").ap()
    woT = nc.dram_tensor("woT", [NHC * HD, HID], bf16, kind="ExternalInput").ap()
    cosQ = nc.dram_tensor("cosQ", [128, BS], bf16, kind="ExternalInput").ap()
    sinQ = nc.dram_tensor("sinQ", [128, BS], bf16, kind="ExternalInput").ap()
    ident = nc.dram_tensor("ident", [128, 64], bf16, kind="ExternalInput").ap()
    identf = nc.dram_tensor("identf", [JT, JT], bf16, kind="ExternalInput").ap()
    maskadd = nc.dram_tensor("maskadd", [JT, JT], bf16, kind="ExternalInput").ap()
    if timing:
        y = None
        ytiny = nc.dram_tensor("ytiny", [1, 8], f32, kind="ExternalOutput").ap()
    else:
        y = nc.dram_tensor("y", [BS, HID], bf16, kind="ExternalOutput").ap()

    xTr = xT.rearrange("(kt p) m -> p kt m", p=128)      # [128, KT, BS]
    wqTr = wqT.rearrange("(kt p) n -> p kt n", p=128)    # [128, KT, 256]
    wkvTr = wkvT.rearrange("(kt p) n -> p kt n", p=128)  # [128, KT, 128]
    woTr = woT.rearrange("(kt p) n -> p kt n", p=128)    # [128, 2, HID]

    with tile.TileContext(nc) as tc:
        with contextlib.ExitStack() as ctx:
            singles = ctx.enter_context(tc.tile_pool(name="singles", bufs=1))
            xpool = ctx.enter_context(tc.tile_pool(name="xpool", bufs=3))
            cospool = ctx.enter_context(tc.tile_pool(name="cospool", bufs=2))
            kvpool = ctx.enter_context(tc.tile_pool(name="kvpool", bufs=2))
            qpool = ctx.enter_context(tc.tile_pool(name="qpool", bufs=2))
            rpool = ctx.enter_context(tc.tile_pool(name="rpool", bufs=3))
            ptpool = ctx.enter_context(tc.tile_pool(name="ptpool", bufs=4))
            opool = ctx.enter_context(tc.tile_pool(name="opool", bufs=2))
            otpool = ctx.enter_context(tc.tile_pool(name="otpool", bufs=2))
            ypool = ctx.enter_context(tc.tile_pool(name="ypool", bufs=3))
            psA = ctx.enter_context(tc.tile_pool(name="psA", bufs=2, space="PSUM"))
            psS = ctx.enter_context(tc.tile_pool(name="psS", bufs=2, space="PSUM"))
            psO = ctx.enter_context(tc.tile_pool(name="psO", bufs=2, space="PSUM"))
            psY = ctx.enter_context(tc.tile_pool(name="psY", bufs=2, space="PSUM"))
            if timing:
                ydram = ctx.enter_context(
                    tc.tile_pool(name="ydram", bufs=1, space="DRAM"))
                y_scratch = ydram.tile([BS, HID], bf16)
                yt_s = None

            # ---- persistent weights / constants ----
            wq_s = singles.tile([128, KT, NHC * HD], bf16)
            nc.sync.dma_start(out=wq_s[:, 0:KT // 2], in_=wqTr[:, 0:KT // 2])
            nc.sync.dma_start(out=wq_s[:, KT // 2:KT], in_=wqTr[:, KT // 2:KT])
            wkv_s = singles.tile([128, KT, 2 * HD], bf16)
            wo_s = singles.tile([128, 2, HID], bf16)
            id_s = singles.tile([128, 64], bf16)
            idf_s = singles.tile([JT, JT], bf16)
            mask_s = singles.tile([JT, JT], bf16)

            _yt = [None]

            def emit_oproj(ot, base, blk):
                """One 128-row output-projection chunk for a finished i-tile."""
                m0 = base + blk * 128
                ys = ypool.tile([128, HID], bf16, tag="ys")
                for n4 in range(HID // IT):
                    p_y = psY.tile([128, IT], f32, tag="py")
                    for kt2 in range(2):
                        nc.tensor.matmul(
                            p_y,
                            ot[:, kt2, blk * 128:(blk + 1) * 128],
                            wo_s[:, kt2, n4 * IT:(n4 + 1) * IT],
                            start=(kt2 == 0), stop=(kt2 == 1))
                    nc.vector.tensor_copy(
                        ys[:, n4 * IT:(n4 + 1) * IT], p_y)
                ytgt = y_scratch if timing else y
                nc.sync.dma_start(out=ytgt[m0:m0 + 128, :], in_=ys)
                if timing and _yt[0] is None:
                    _yt[0] = ypool.tile([1, 8], f32, tag="yt", name="yt_s")
                    nc.vector.tensor_copy(_yt[0], ys[0:1, 0:8])
                    nc.sync.dma_start(out=ytiny, in_=_yt[0])

            pend_C = None
            for rep in range(repeat):
              for b in range(B):
                cb = b * S
                xt0 = xpool.tile([128, KT, IT], bf16, tag="xt")
                nc.sync.dma_start(
                    out=xt0[:, 0:KT // 2], in_=xTr[:, 0:KT // 2, cb:cb + IT])
                nc.sync.dma_start(
                    out=xt0[:, KT // 2:KT], in_=xTr[:, KT // 2:KT, cb:cb + IT])
                if rep == 0 and b == 0:
                    nc.sync.dma_start(out=wkv_s, in_=wkvTr)
                cq_s = cospool.tile([128, S], bf16, tag="cq")
                sq_s = cospool.tile([128, S], bf16, tag="sq")
                nc.sync.dma_start(out=cq_s, in_=cosQ[:, cb:cb + S])
                nc.sync.dma_start(out=sq_s, in_=sinQ[:, cb:cb + S])
                if rep == 0 and b == 0:
                    nc.sync.dma_start(out=id_s, in_=ident)
                    nc.sync.dma_start(out=idf_s, in_=identf)
                    nc.sync.dma_start(out=mask_s, in_=maskadd)
                    nc.sync.dma_start(out=wo_s, in_=woTr)

                # per-batch K (by i-tile) and V (seq-major with ones cols)
                kts = [kvpool.tile([64, IT], bf16, tag=f"kT{i}",
                                   name=f"kT{i}_{b}_{rep}") for i in range(NII)]
                vt = kvpool.tile([128, NJ, JT], bf16, tag="vt",
                                 name=f"vt_{b}_{rep}")
                nc.vector.memset(vt[:, :, HD:JT], 1.0)

                for ii in range(NII):
                    i0 = ii * IT
                    isl = slice(i0, i0 + IT)
                    if ii == 0:
                        xt = xt0
                    else:
                        xt = xpool.tile([128, KT, IT], bf16, tag="xt")
                        nc.sync.dma_start(
                            out=xt[:, 0:KT // 2],
                            in_=xTr[:, 0:KT // 2, cb + i0:cb + i0 + IT])
                        nc.sync.dma_start(
                            out=xt[:, KT // 2:KT],
                            in_=xTr[:, KT // 2:KT, cb + i0:cb + i0 + IT])

                    qt = qpool.tile([64, NHC, IT], bf16, tag="qt")

                    # -- Q projection + RoPE (2 groups of 2 heads) --
                    for ni in range(2):
                        p_q = psA.tile([128, IT], f32, tag="pa")
                        for kt in range(KT):
                            nc.tensor.matmul(
                                p_q, wq_s[:, kt, ni * 128:(ni + 1) * 128],
                                xt[:, kt, :],
                                start=(kt == 0), stop=(kt == KT - 1))
                        q_sb = rpool.tile([128, IT], bf16, tag="qraw")
                        nc.scalar.copy(q_sb, p_q)
                        t_c = rpool.tile([128, IT], bf16, tag="tc")
                        t_s = rpool.tile([128, IT], bf16, tag="ts")
                        nc.vector.tensor_mul(t_c, q_sb, cq_s[:, isl])
                        for r0 in (0, 64):
                            nc.vector.tensor_mul(
                                t_s[r0:r0 + 32], q_sb[r0 + 32:r0 + 64],
                                sq_s[r0 + 32:r0 + 64, isl])
                            nc.vector.tensor_mul(
                                t_s[r0 + 32:r0 + 64], q_sb[r0:r0 + 32],
                                sq_s[r0:r0 + 32, isl])
                        nc.vector.tensor_add(
                            qt[:, 2 * ni, :], t_c[0:64], t_s[0:64])
                        nc.vector.tensor_add(
                            qt[:, 2 * ni + 1, :], t_c[64:128], t_s[64:128])

                    # -- K/V projection; K RoPE; V transpose --
                    p_kv = psA.tile([128, IT], f32, tag="pa")
                    for kt in range(KT):
                        nc.tensor.matmul(
                            p_kv, wkv_s[:, kt, :], xt[:, kt, :],
                            start=(kt == 0), stop=(kt == KT - 1))
                    kv_sb = rpool.tile([128, IT], bf16, tag="qraw")
                    nc.scalar.copy(kv_sb, p_kv)
                    t_c = rpool.tile([128, IT], bf16, tag="tc")
                    t_s = rpool.tile([128, IT], bf16, tag="ts")
                    nc.vector.tensor_mul(t_c[0:64], kv_sb[0:64], cq_s[0:64, isl])
                    nc.vector.tensor_mul(
                        t_s[0:32], kv_sb[32:64], sq_s[32:64, isl])
                    nc.vector.tensor_mul(
                        t_s[32:64], kv_sb[0:32], sq_s[0:32, isl])
                    nc.vector.tensor_add(kts[ii][:, :], t_c[0:64], t_s[0:64])
                    for j2 in range(IT // JT):
                        p_v = psO.tile([128, 64], bf16, tag="po")
                        nc.tensor.transpose(
                            p_v, kv_sb[64:128, j2 * JT:(j2 + 1) * JT],
                            id_s[64:128, :])
                        nc.vector.tensor_copy(
                            vt[:, (i0 // JT) + j2, 0:HD], p_v)

                    # -- attention for this i-tile, with the previous
                    #    i-tile's output projection interleaved per head to
                    #    fill PE stalls at head transitions --
                    ot = otpool.tile([128, 2, IT], bf16, tag="ot")
                    jmax = 4 * ii + 3
                    for h in range(NHC):
                        p_o = psO.tile([128, IT], f32, tag="po")
                        for J in range(jmax + 1):
                            Jii, Jr = J // 4, J % 4
                            r = J - 4 * ii
                            c0 = r * JT if r > 0 else 0
                            p_s = psS.tile([128, IT], f32, tag="ps")
                            nc.tensor.matmul(
                                p_s[:, c0:IT],
                                kts[Jii][:, Jr * JT:(Jr + 1) * JT],
                                qt[:, h, c0:IT],
                                start=True, stop=(r < 0),
                                skip_group_check=True)
                            if r >= 0:
                                nc.tensor.matmul(
                                    p_s[:, c0:c0 + JT], idf_s, mask_s,
                                    start=False, stop=True,
                                    skip_group_check=True)
                            pt = ptpool.tile([128, IT], bf16, tag="pt")
                            nc.scalar.activation(
                                pt[:, c0:IT], p_s[:, c0:IT], EXP)
                            nc.tensor.matmul(
                                p_o[:, c0:IT], vt[:, J, :], pt[:, c0:IT],
                                start=(J == 0), stop=(J == jmax),
                                skip_group_check=True)
                        # normalize: rows 64:128 of p_o are the rowsum
                        # (broadcast via the ones columns of vt)
                        osb = opool.tile([128, IT], bf16, tag="osb")
                        nc.vector.tensor_copy(osb, p_o)
                        rcr = opool.tile([64, IT], bf16, tag="rcr")
                        with nc.allow_low_precision(reason="bf16 softmax recip"):
                            nc.vector.reciprocal(rcr, osb[64:128])
                        ntile, hr = h // 2, (h % 2) * 64
                        nc.vector.tensor_mul(
                            ot[hr:hr + 64, ntile, :], osb[0:HD], rcr)
                        if pend_C is not None:
                            emit_oproj(pend_C[0], pend_C[1], h)
                    pend_C = (ot, cb + i0)

            if pend_C is not None:
                for blk in range(IT // 128):
                    emit_oproj(pend_C[0], pend_C[1], blk)

    nc.compile()
    return nc


# --------------------------------------------------------------------------
# general (arbitrary mask) fallback: f32r kernel
# --------------------------------------------------------------------------

def _build_general(repeat=1, timing=False):
    import contextlib
    import concourse.bacc as bacc
    import concourse.tile as tile
    import concourse.mybir as mybir

    f32 = mybir.dt.float32
    f32r = mybir.dt.float32r
    bf16 = mybir.dt.bfloat16
    EXP = mybir.ActivationFunctionType.Exp

    nc = bacc.Bacc("TRN2", target_bir_lowering=False, debug=False)

    xT = nc.dram_tensor("xT", [HID, BS], f32, kind="ExternalInput").ap()
    wqT = nc.dram_tensor("wqT", [HID, NHC * HD], f32, kind="ExternalInput").ap()
    wkvT = nc.dram_tensor("wkvT", [HID, 2 * HD], f32, kind="ExternalInput").ap()
    woT = nc.dram_tensor("woT", [NHC * HD, HID], f32, kind="ExternalInput").ap()
    cosQ = nc.dram_tensor("cosQ", [128, BS], f32, kind="ExternalInput").ap()
    sinQ = nc.dram_tensor("sinQ", [128, BS], f32, kind="ExternalInput").ap()
    identhi = nc.dram_tensor("identhi", [128, 64], f32, kind="ExternalInput").ap()
    onesd = nc.dram_tensor("onesd", [128, 64], f32, kind="ExternalInput").ap()
    maskT = nc.dram_tensor("maskT", [S, BS], bf16, kind="ExternalInput").ap()
    maskTr = maskT.rearrange("(J p) i -> p J i", p=128)
    if timing:
        y = None
        ytiny = nc.dram_tensor("ytiny", [1, 8], f32, kind="ExternalOutput").ap()
    else:
        y = nc.dram_tensor("y", [BS, HID], f32, kind="ExternalOutput").ap()

    xTr = xT.rearrange("(kt p) m -> p kt m", p=128)
    wqTr = wqT.rearrange("(kt p) n -> p kt n", p=128)
    wkvTr = wkvT.rearrange("(kt p) n -> p kt n", p=128)
    woTr = woT.rearrange("(kt p) n -> p kt n", p=128)

    NMI = S // XCHUNK

    with tile.TileContext(nc) as tc:
        with contextlib.ExitStack() as ctx:
            singles = ctx.enter_context(tc.tile_pool(name="singles", bufs=1))
            xpool = ctx.enter_context(tc.tile_pool(name="xpool", bufs=3))
            cospool = ctx.enter_context(tc.tile_pool(name="cospool", bufs=1))
            qkv = ctx.enter_context(tc.tile_pool(name="qkv", bufs=1))
            ropetmp = ctx.enter_context(tc.tile_pool(name="ropetmp", bufs=1))
            ppool = ctx.enter_context(tc.tile_pool(name="ppool", bufs=3))
            nrm = ctx.enter_context(tc.tile_pool(name="nrm", bufs=1))
            ypool = ctx.enter_context(tc.tile_pool(name="ypool", bufs=2))
            mpool = ctx.enter_context(tc.tile_pool(name="mpool", bufs=1))
            pa_ps = ctx.enter_context(tc.tile_pool(name="pa_ps", bufs=2, space="PSUM"))
            mm_ps = ctx.enter_context(tc.tile_pool(name="mm_ps", bufs=2, space="PSUM"))
            o_ps = ctx.enter_context(tc.tile_pool(name="o_ps", bufs=2, space="PSUM"))
            v_ps = ctx.enter_context(tc.tile_pool(name="v_ps", bufs=1, space="PSUM"))
            b_ps = ctx.enter_context(tc.tile_pool(name="b_ps", bufs=1, space="PSUM"))
            if timing:
                ydram = ctx.enter_context(
                    tc.tile_pool(name="ydram", bufs=1, space="DRAM"))
                y_scratch = ydram.tile([BS, HID], f32)
                yt_s = None

            wq_s = singles.tile([128, KT, NHC * HD], f32r)
            nc.sync.dma_start(out=wq_s, in_=wqTr.bitcast(f32r))
            wkv_s = singles.tile([128, KT, 2 * HD], f32r)
            nc.sync.dma_start(out=wkv_s, in_=wkvTr.bitcast(f32r))
            wo_s = singles.tile([128, 2, HID], f32r)
            nc.sync.dma_start(out=wo_s, in_=woTr.bitcast(f32r))
            ident_hi = singles.tile([128, 64], f32)
            nc.sync.dma_start(out=ident_hi, in_=identhi)
            ones_t = singles.tile([128, 64], f32r)
            nc.sync.dma_start(out=ones_t, in_=onesd.bitcast(f32r))

            for rep in range(repeat):
              for b in range(B):
                cb = b * S
                cq_s = cospool.tile([128, S], f32, tag="cq")
                sq_s = cospool.tile([128, S], f32, tag="sq")
                nc.sync.dma_start(out=cq_s, in_=cosQ[:, cb:cb + S])
                nc.sync.dma_start(out=sq_s, in_=sinQ[:, cb:cb + S])

                qT4 = [qkv.tile([64, NHC, IT], f32r, tag=f"qT{i}",
                                name=f"qT{i}_{b}") for i in range(NII)]
                kT4 = [qkv.tile([64, IT], f32r, tag=f"kT{i}", bufs=1,
                                name=f"kT{i}_{b}") for i in range(NII)]
                v4 = [qkv.tile([128, IT // JT, HD + 1], f32r, tag=f"v{i}", bufs=1,
                               name=f"v{i}_{b}") for i in range(NII)]
                for i in range(NII):
                    nc.sync.dma_start(
                        out=v4[i][:, :, HD:HD + 1],
                        in_=onesd[:, 0:IT // JT]
                        .rearrange("p (a b) -> p a b", b=1).bitcast(f32r))
                oT4 = [qkv.tile([128, 2, IT], f32r, tag=f"oT{i}",
                                name=f"oT{i}_{b}") for i in range(NII)]

                for mi in range(NMI):
                    m0 = mi * XCHUNK
                    msl = slice(m0, m0 + XCHUNK)
                    mii = m0 // IT
                    l0 = m0 % IT
                    lsl = slice(l0, l0 + XCHUNK)
                    xt = xpool.tile([128, KT, XCHUNK], f32r, tag="xt")
                    nc.sync.dma_start(
                        out=xt, in_=xTr[:, :, cb + m0:cb + m0 + XCHUNK].bitcast(f32r))

                    for ni in range(2):
                        p_q = pa_ps.tile([128, IT], f32, tag="pa")
                        for kt in range(KT):
                            nc.tensor.matmul(
                                p_q[:, :XCHUNK],
                                wq_s[:, kt, ni * 128:(ni + 1) * 128],
                                xt[:, kt, :],
                                start=(kt == 0), stop=(kt == KT - 1))
                        q_raw = ropetmp.tile([128, XCHUNK], f32, tag="qraw")
                        nc.scalar.copy(q_raw, p_q[:, :XCHUNK])
                        t_c = ropetmp.tile([128, XCHUNK], f32, tag="tc")
                        t_s = ropetmp.tile([128, XCHUNK], f32, tag="ts")
                        nc.vector.tensor_mul(t_c, q_raw, cq_s[:, msl])
                        for r0 in (0, 64):
                            nc.vector.tensor_mul(
                                t_s[r0:r0 + 32], q_raw[r0 + 32:r0 + 64],
                                sq_s[r0 + 32:r0 + 64, msl])
                            nc.vector.tensor_mul(
                                t_s[r0 + 32:r0 + 64], q_raw[r0:r0 + 32],
                                sq_s[r0:r0 + 32, msl])
                        nc.vector.tensor_add(
                            qT4[mii][:, 2 * ni, lsl], t_c[0:64], t_s[0:64])
                        nc.vector.tensor_add(
                            qT4[mii][:, 2 * ni + 1, lsl], t_c[64:128], t_s[64:128])

                    p_kv = pa_ps.tile([128, IT], f32, tag="pa")
                    for kt in range(KT):
                        nc.tensor.matmul(
                            p_kv[:, :XCHUNK], wkv_s[:, kt, :], xt[:, kt, :],
                            start=(kt == 0), stop=(kt == KT - 1))
                    kv_raw = ropetmp.tile([128, XCHUNK], f32, tag="qraw")
                    nc.scalar.copy(kv_raw, p_kv[:, :XCHUNK])
                    t_c = ropetmp.tile([128, XCHUNK], f32, tag="tc")
                    t_s = ropetmp.tile([128, XCHUNK], f32, tag="ts")
                    nc.vector.tensor_mul(t_c[0:64], kv_raw[0:64], cq_s[0:64, msl])
                    nc.vector.tensor_mul(
                        t_s[0:32], kv_raw[32:64], sq_s[32:64, msl])
                    nc.vector.tensor_mul(
                        t_s[32:64], kv_raw[0:32], sq_s[0:32, msl])
                    nc.vector.tensor_add(kT4[mii][:, lsl], t_c[0:64], t_s[0:64])
                    for jj2 in range(XCHUNK // JT):
                        jt = (l0 // JT) + jj2
                        p_v = v_ps.tile([128, 64], f32, tag="vt")
                        nc.tensor.transpose(
                            p_v, kv_raw[64:128, jj2 * JT:(jj2 + 1) * JT],
                            ident_hi[64:128, :])
                        nc.vector.tensor_copy(v4[mii][:, jt, 0:HD], p_v)

                for ii in range(NII):
                    i0 = ii * IT
                    jmax = NJ - 1
                    mk_s = mpool.tile([128, NJ, IT], bf16, tag="mk")
                    nc.sync.dma_start(
                        out=mk_s, in_=maskTr[:, :, cb + i0:cb + i0 + IT])
                    for h in range(NHC):
                        p_o = o_ps.tile([HD + 1, IT], f32, tag="po")
                        for J in range(jmax + 1):
                            Jii, Jr = J // (IT // JT), J % (IT // JT)
                            ksl = kT4[Jii][:, Jr * JT:(Jr + 1) * JT]
                            pt = ppool.tile([128, IT], f32r, tag="pt")
                            p_s = mm_ps.tile([128, IT], f32, tag="mm")
                            nc.tensor.matmul(
                                p_s, ksl, qT4[ii][:, h, :],
                                start=True, stop=True)
                            nc.vector.tensor_add(p_s, p_s, mk_s[:, J, :])
                            nc.scalar.activation(pt, p_s, EXP)
                            nc.tensor.matmul(
                                p_o, v4[Jii][:, Jr, :], pt,
                                start=(J == 0), stop=(J == jmax),
                                skip_group_check=True)
                        rcr_t = nrm.tile([65, IT], f32r, tag="rcr")
                        with nc.allow_low_precision(reason="f32r rowsum recip"):
                            nc.vector.reciprocal(rcr_t[64:65, :], p_o[HD:HD + 1, :])
                        p_b = b_ps.tile([64, IT], f32, tag="pb")
                        nc.tensor.matmul(
                            p_b, ones_t[64:65, :], rcr_t[64:65, :],
                            start=True, stop=True)
                        rb_s = nrm.tile([64, IT], f32, tag="rb")
                        nc.scalar.copy(rb_s, p_b)
                        ntile, hr = h // 2, (h % 2) * 64
                        if hr == 0:
                            nc.vector.tensor_mul(
                                oT4[ii][0:64, ntile, :], p_o[0:HD, :], rb_s)
                        else:
                            otmp = nrm.tile([64, IT], f32r, tag="otmp")
                            nc.vector.tensor_mul(otmp, p_o[0:HD, :], rb_s)
                            nc.vector.tensor_copy(
                                oT4[ii][64:128, ntile, :], otmp)

                for mi2 in range(S // 128):
                    m0 = mi2 * 128
                    mii2 = m0 // IT
                    lm0 = m0 % IT
                    for nh2 in range(2):
                        ys = ypool.tile([128, HID // 2], f32, tag="ys")
                        for ni2 in range(2):
                            n0 = nh2 * (HID // 2) + ni2 * IT
                            p_y = mm_ps.tile([128, IT], f32, tag="mm")
                            for kt2 in range(2):
                                nc.tensor.matmul(
                                    p_y, oT4[mii2][:, kt2, lm0:lm0 + 128],
                                    wo_s[:, kt2, n0:n0 + IT],
                                    start=(kt2 == 0), stop=(kt2 == 1))
                            if (mi2 + ni2) % 2 == 0:
                                nc.vector.tensor_copy(
                                    ys[:, ni2 * IT:(ni2 + 1) * IT], p_y)
                            else:
                                nc.scalar.copy(ys[:, ni2 * IT:(ni2 + 1) * IT], p_y)
                        ytgt = y_scratch if timing else y
                        nc.sync.dma_start(
                            out=ytgt[cb + m0:cb + m0 + 128,
                                     nh2 * (HID // 2):(nh2 + 1) * (HID // 2)],
                            in_=ys)
                        if timing and yt_s is None:
                            yt_s = ypool.tile([1, 8], f32, tag="yt")
                            nc.vector.tensor_copy(yt_s, ys[0:1, 0:8])
                            nc.sync.dma_start(out=ytiny, in_=yt_s)

    nc.compile()
    return nc


def _build(mode, repeat=1, timing=False, phases=None):
    if mode == "causal":
        return _build_causal(repeat=repeat, timing=timing)
    return _build_general(repeat=repeat, timing=timing)


def _get_program(mode):
    if mode not in _programs:
        _programs[mode] = _build(mode)
    return _programs[mode]


# --------------------------------------------------------------------------
# host-side prep
# --------------------------------------------------------------------------

def _rope_tables(cos, sin, dtype):
    cosT = np.concatenate([cos[b].T for b in range(B)], axis=1).astype(np.float32)
    sinT = np.concatenate([sin[b].T for b in range(B)], axis=1).astype(np.float32)
    sinS = np.concatenate([sinT[0:HD // 2], -sinT[0:HD // 2]], axis=0)
    cosQ = np.ascontiguousarray(np.concatenate([cosT, cosT], axis=0)).astype(dtype)
    sinQ = np.ascontiguousarray(np.concatenate([sinS, sinS], axis=0)).astype(dtype)
    return cosQ, sinQ


def _make_in_maps_causal(inputs_f32):
    hidden_states, cos, sin, attention_mask, Wq, Wk, Wv, Wo = inputs_f32
    bf16 = _bf16()
    X = np.ascontiguousarray(
        hidden_states.reshape(BS, HID).T).astype(bf16)
    cosQ, sinQ = _rope_tables(cos, sin, bf16)
    identb = np.zeros((128, 64), dtype=np.float32)
    identb[64:128, :] = np.eye(64, dtype=np.float32)
    identb = identb.astype(bf16)
    jj = np.arange(JT, dtype=np.float32)
    madd = np.where(jj[None, :] >= jj[:, None], 0.0, NEG).astype(bf16)
    identf = np.eye(JT, dtype=np.float32).astype(bf16)
    in_maps = []
    for c in range(NCORES):
        wq_c = (Wq[c * NHC * HD:(c + 1) * NHC * HD, :] * SCALE)
        wqT = np.ascontiguousarray(wq_c.T).astype(bf16)
        wk_c = Wk[c * HD:(c + 1) * HD, :]
        wv_c = Wv[c * HD:(c + 1) * HD, :]
        wkvT = np.ascontiguousarray(
            np.concatenate([wk_c, wv_c], axis=0).T).astype(bf16)
        woT = np.ascontiguousarray(
            Wo[:, c * NHC * HD:(c + 1) * NHC * HD].T).astype(bf16)
        in_maps.append({"xT": X, "wqT": wqT, "wkvT": wkvT, "woT": woT,
                        "cosQ": cosQ, "sinQ": sinQ, "ident": identb,
                        "identf": identf, "maskadd": madd})
    return in_maps


def _make_in_maps_general(inputs_f32):
    hidden_states, cos, sin, attention_mask, Wq, Wk, Wv, Wo = inputs_f32
    f32 = np.float32
    X = np.ascontiguousarray(hidden_states.reshape(BS, HID).T).astype(f32, copy=False)
    cosQ, sinQ = _rope_tables(cos, sin, f32)
    identhi = np.zeros((128, 64), dtype=f32)
    identhi[64:128, :] = np.eye(64, dtype=f32)
    onesd = np.ones((128, 64), dtype=f32)
    mT = np.concatenate([attention_mask[b, 0].T for b in range(B)], axis=1)
    mT = np.ascontiguousarray(mT).astype(_bf16())
    in_maps = []
    for c in range(NCORES):
        wq_c = Wq[c * NHC * HD:(c + 1) * NHC * HD, :] * SCALE
        wqT = np.ascontiguousarray(wq_c.T.astype(f32))
        wk_c = Wk[c * HD:(c + 1) * HD, :]
        wv_c = Wv[c * HD:(c + 1) * HD, :]
        wkvT = np.ascontiguousarray(np.concatenate([wk_c, wv_c], axis=0).T.astype(f32))
        woT = np.ascontiguousarray(Wo[:, c * NHC * HD:(c + 1) * NHC * HD].T.astype(f32))
        in_maps.append({"xT": X, "wqT": wqT, "wkvT": wkvT, "woT": woT,
                        "cosQ": cosQ, "sinQ": sinQ, "identhi": identhi,
                        "onesd": onesd, "maskT": mT})
    return in_maps


def _make_in_maps(inputs_f32, causal):
    if causal:
        return _make_in_maps_causal(inputs_f32)
    return _make_in_maps_general(inputs_f32)


def _is_causal(attention_mask):
    am = np.asarray(attention_mask)
    if am.shape != (B, 1, S, S):
        return False
    tri = np.where(np.tril(np.ones((S, S), dtype=bool)),
                   np.float32(0.0), np.float32(NEG))
    return bool(np.array_equal(am[0, 0], tri) and np.array_equal(am[1, 0], tri))


def kernel(hidden_states, cos, sin, attention_mask, Wq, Wk, Wv, Wo):
    from concourse.bass_utils import run_bass_kernel_spmd

    inputs_f32 = tuple(
        np.asarray(a, dtype=np.float32)
        for a in (hidden_states, cos, sin, attention_mask, Wq, Wk, Wv, Wo))

    causal = _is_causal(inputs_f32[3])
    nc = _get_program("causal" if causal else "general")
    in_maps = _make_in_maps(inputs_f32, causal)

    res = run_bass_kernel_spmd(nc, in_maps, core_ids=list(range(NCORES)))
    acc = np.zeros((BS, HID), dtype=np.float32)
    for c in range(NCORES):
        acc += res.results[c]["y"].astype(np.float32)
    return acc.reshape(B, S, HID)
